# revision 1
# baseline (speedup 1.0000x reference)
"""GraphSAGE 2-layer forward on 8 Trainium2 NeuronCores.

Strategy (dst-range sharding):
  - Core c owns destination nodes [c*NPC, (c+1)*NPC). It receives ALL edges
    whose dst lands in its range, so local segment-sums are exact (no
    all-reduce). One AllGather exchanges the hidden layer between layers.
  - Edges are host-sorted by destination into 128-node windows. Per window,
    messages x[src] are DMA-gathered (256B rows) into edge-major SBUF tiles
    [128 edges, 64 feats]. A weighted one-hot (iota==dstloc)*(1/deg) built in
    one DVE op turns the per-window segment-mean into PE matmuls accumulating
    into PSUM (feature-major mean^T [64, 128 nodes]).
  - dma_gather indices are int16 (<32768), so edges are split into two
    passes: src < 32768 (table base 0) and src >= 32768 (table base shifted).
    Both passes accumulate into the same PSUM window.
  - Dense part: h^T = relu(W_l @ mean^T + W_r @ x^T + b) stays feature-major;
    PE transpose writes node-major h to DRAM for the layer-2 gather.
"""

import numpy as np

import concourse.bass as bass
import concourse.bacc as bacc
import concourse.tile as tile
from concourse import mybir
from concourse.bass_utils import run_bass_kernel_spmd
from concourse.masks import make_identity

F32 = mybir.dt.float32
I16 = mybir.dt.int16

# Problem constants (hardcoded per contract)
N = 50000
E = 800000
F = 64
HID = 64
OUT = 2
NCORES = 8
NPC = N // NCORES          # 6250 nodes per core
WIN = 128                  # nodes per window (one PSUM bank width)
NW = (NPC + WIN - 1) // WIN  # 49 windows per core
NPC_PAD = NW * WIN         # 6272
SPLIT = 32768              # int16 index limit
CHUNK_TILES = 96             # max message tiles gathered per chunk


def _plan_edges(edge_index):
    """Host-side graph preprocessing: per-core, per-window, per-pass edge
    slotting. Returns compile-time tile plan + per-core device arrays."""
    src = edge_index[0].astype(np.int64)
    dst = edge_index[1].astype(np.int64)
    deg = np.bincount(dst, minlength=N)
    wrec = (1.0 / np.maximum(deg, 1)).astype(np.float32)

    core = dst // NPC
    loc = dst % NPC
    win = loc // WIN
    locw = loc % WIN
    pas = (src >= SPLIT).astype(np.int64)

    key = (core * NW + win) * 2 + pas
    cnt = np.bincount(key, minlength=NCORES * NW * 2).reshape(NCORES, NW, 2)
    # tiles per (window, pass), shared across cores (same compiled program)
    tiles = np.maximum(1, -(-cnt.max(axis=0) // 128))  # [NW, 2]
    tA = tiles[:, 0]
    tB = tiles[:, 1]
    TA, TB = int(tA.sum()), int(tB.sum())
    a0 = np.concatenate([[0], np.cumsum(tA)])  # A-tile offsets per window
    b0 = np.concatenate([[0], np.cumsum(tB)])
    LA, LB = TA * 128, TB * 128

    order = np.argsort(key, kind="stable")

    per_core = []
    for c in range(NCORES):
        arrs = {}
        for p, (Tn, base, L) in enumerate(((tA, a0, LA), (tB, b0, LB))):
            idx_flat = np.zeros(L, np.int16)
            dl_flat = np.full(L, 210.0, np.float32)
            wv_flat = np.zeros(L, np.float32)
            for w in range(NW):
                k = (c * NW + w) * 2 + p
                s0 = int(np.searchsorted(key[order], k))
                s1 = int(np.searchsorted(key[order], k + 1))
                ed = order[s0:s1]
                j = base[w] * 128 + np.arange(len(ed))
                sv = src[ed] - (SPLIT if p else 0)
                idx_flat[j] = sv.astype(np.int16)
                dl_flat[j] = locw[ed].astype(np.float32)
                wv_flat[j] = wrec[dst[ed]]
            tag = "AB"[p]
            arrs[f"idx{tag}"] = np.ascontiguousarray(
                np.tile(idx_flat.reshape(L // 16, 16).T, (8, 1)))
            arrs[f"dl{tag}"] = np.ascontiguousarray(
                dl_flat.reshape(-1, 128).T)
            arrs[f"wv{tag}"] = np.ascontiguousarray(
                wv_flat.reshape(-1, 128).T)
        per_core.append(arrs)

    # chunk windows so that each chunk's message tiles fit SBUF
    chunks = []
    cur = []
    cur_t = 0
    for w in range(NW):
        t = int(tA[w] + tB[w])
        if cur and cur_t + t > CHUNK_TILES:
            chunks.append(cur)
            cur = []
            cur_t = 0
        cur.append(w)
        cur_t += t
    if cur:
        chunks.append(cur)

    plan = dict(tA=tA.tolist(), tB=tB.tolist(),
                a0=a0.tolist(), b0=b0.tolist(),
                TA=TA, TB=TB, chunks=chunks)
    return plan, per_core


def _build(plan, collective=True, layers=2):
    """Build the SPMD Bass program (same for all cores)."""
    tA, tB = plan["tA"], plan["tB"]
    a0, b0 = plan["a0"], plan["b0"]
    TA, TB = plan["TA"], plan["TB"]
    chunks = plan["chunks"]
    maxA = max(sum(tA[w] for w in ch) for ch in chunks)
    maxB = max(sum(tB[w] for w in ch) for ch in chunks)

    nc = bacc.Bacc("TRN2", target_bir_lowering=False, debug=False,
                   num_devices=NCORES)

    x_d = nc.dram_tensor("x", [N, F], F32, kind="ExternalInput")
    xT_d = nc.dram_tensor("xT", [F, NPC_PAD], F32, kind="ExternalInput")
    idxA_d = nc.dram_tensor("idxA", [128, TA * 8], I16, kind="ExternalInput")
    idxB_d = nc.dram_tensor("idxB", [128, TB * 8], I16, kind="ExternalInput")
    dlA_d = nc.dram_tensor("dlA", [128, TA], F32, kind="ExternalInput")
    wvA_d = nc.dram_tensor("wvA", [128, TA], F32, kind="ExternalInput")
    dlB_d = nc.dram_tensor("dlB", [128, TB], F32, kind="ExternalInput")
    wvB_d = nc.dram_tensor("wvB", [128, TB], F32, kind="ExternalInput")
    w1l_d = nc.dram_tensor("W1lT", [F, HID], F32, kind="ExternalInput")
    w1r_d = nc.dram_tensor("W1rT", [F, HID], F32, kind="ExternalInput")
    w2l_d = nc.dram_tensor("W2lT", [HID, OUT], F32, kind="ExternalInput")
    w2r_d = nc.dram_tensor("W2rT", [HID, OUT], F32, kind="ExternalInput")
    b1_d = nc.dram_tensor("b1", [HID, 1], F32, kind="ExternalInput")
    b2_d = nc.dram_tensor("b2", [OUT, 1], F32, kind="ExternalInput")
    iota_d = nc.dram_tensor("iota", [128, 128], F32, kind="ExternalInput")
    outT_d = nc.dram_tensor("outT", [OUT, NPC_PAD], F32, kind="ExternalOutput")

    h_shard = nc.dram_tensor("h_shard", [NPC, HID], F32)
    h_full = nc.dram_tensor("h_full", [N, HID], F32,
                            addr_space="Shared" if collective else "Local")

    with tile.TileContext(nc) as tc:
        with (
            tc.tile_pool(name="const", bufs=1) as cpool,
            tc.tile_pool(name="msg", bufs=2) as mpool,
            tc.tile_pool(name="oh", bufs=4) as ohpool,
            tc.tile_pool(name="small", bufs=3) as spool,
            tc.tile_pool(name="agg", bufs=4, space="PSUM") as aggp,
            tc.tile_pool(name="dense", bufs=2, space="PSUM") as densep,
            tc.tile_pool(name="tp", bufs=2, space="PSUM") as tpp,
        ):
            # ---- constants to SBUF
            iota = cpool.tile([128, 128], F32)
            nc.sync.dma_start(out=iota[:], in_=iota_d[:])
            ident = cpool.tile([128, 128], F32)
            make_identity(nc, ident[:])
            w1l = cpool.tile([F, HID], F32)
            nc.sync.dma_start(out=w1l[:], in_=w1l_d[:])
            w1r = cpool.tile([F, HID], F32)
            nc.sync.dma_start(out=w1r[:], in_=w1r_d[:])
            w2l = cpool.tile([HID, OUT], F32)
            nc.sync.dma_start(out=w2l[:], in_=w2l_d[:])
            w2r = cpool.tile([HID, OUT], F32)
            nc.sync.dma_start(out=w2r[:], in_=w2r_d[:])
            b1 = cpool.tile([HID, 1], F32)
            nc.sync.dma_start(out=b1[:], in_=b1_d[:])
            b2 = cpool.tile([OUT, 1], F32)
            nc.sync.dma_start(out=b2[:], in_=b2_d[:])
            xT = cpool.tile([F, NPC_PAD], F32)
            nc.sync.dma_start(out=xT[:], in_=xT_d[:])
            idxA = cpool.tile([128, TA * 8], I16)
            nc.sync.dma_start(out=idxA[:], in_=idxA_d[:])
            idxB = cpool.tile([128, TB * 8], I16)
            nc.sync.dma_start(out=idxB[:], in_=idxB_d[:])
            dlA = cpool.tile([128, TA], F32)
            nc.sync.dma_start(out=dlA[:], in_=dlA_d[:])
            wvA = cpool.tile([128, TA], F32)
            nc.sync.dma_start(out=wvA[:], in_=wvA_d[:])
            dlB = cpool.tile([128, TB], F32)
            nc.sync.dma_start(out=dlB[:], in_=dlB_d[:])
            wvB = cpool.tile([128, TB], F32)
            nc.sync.dma_start(out=wvB[:], in_=wvB_d[:])

            hT = cpool.tile([HID, NPC_PAD], F32)
            outT = cpool.tile([OUT, NPC_PAD], F32)

            for layer in range(layers):
                for ch in chunks:
                    w_lo, w_hi = ch[0], ch[-1] + 1
                    ca0, ca1 = a0[w_lo], a0[w_hi]
                    cb0, cb1 = b0[w_lo], b0[w_hi]
                    nta, ntb = ca1 - ca0, cb1 - cb0
                    msgA = mpool.tile([128, maxA * F], F32, tag="msgA")
                    msgB = mpool.tile([128, maxB * F], F32, tag="msgB")
                    for (msg, nt, cc0, idx, base) in (
                        (msgA, nta, ca0, idxA, 0),
                        (msgB, ntb, cb0, idxB, SPLIT),
                    ):
                        table = x_d if layer == 0 else h_full
                        nc.gpsimd.dma_gather(
                            out_ap=msg[:, :nt * F].rearrange(
                                "p (t f) -> p t f", f=F),
                            in_ap=table[base:, :],
                            idxs_ap=idx[:, cc0 * 8:(cc0 + nt) * 8],
                            num_idxs=nt * 128,
                            num_idxs_reg=nt * 128,
                            elem_size=F,
                            single_packet=False,
                        )
                    for w in ch:
                        psum = aggp.tile([F, 128], F32, tag="agg")
                        work = (
                            [(msgA, t - ca0, dlA, wvA, t)
                             for t in range(a0[w], a0[w + 1])]
                            + [(msgB, t - cb0, dlB, wvB, t)
                               for t in range(b0[w], b0[w + 1])]
                        )
                        for i, (msg, mc, dl, wv, t) in enumerate(work):
                            oh = ohpool.tile([128, 128], F32, tag="oh")
                            nc.vector.tensor_scalar(
                                out=oh[:],
                                in0=iota[:],
                                scalar1=dl[:, t:t + 1],
                                scalar2=wv[:, t:t + 1],
                                op0=mybir.AluOpType.is_equal,
                                op1=mybir.AluOpType.mult,
                            )
                            nc.tensor.matmul(
                                out=psum[:],
                                lhsT=msg[:, mc * F:(mc + 1) * F],
                                rhs=oh[:],
                                start=(i == 0),
                                stop=(i == len(work) - 1),
                            )
                        meanT = spool.tile([F, 128], F32, tag="meanT")
                        nc.vector.tensor_copy(out=meanT[:], in_=psum[:])
                        odim = HID if layer == 0 else OUT
                        dps = densep.tile([odim, 128], F32, tag="dense")
                        wl, wr = (w1l, w1r) if layer == 0 else (w2l, w2r)
                        selfT = xT if layer == 0 else hT
                        nc.tensor.matmul(out=dps[:], lhsT=wl[:], rhs=meanT[:],
                                         start=True, stop=False)
                        nc.tensor.matmul(out=dps[:], lhsT=wr[:],
                                         rhs=selfT[:, w * 128:(w + 1) * 128],
                                         start=False, stop=True)
                        cols = slice(w * 128, (w + 1) * 128)
                        if layer == 0:
                            nc.scalar.activation(
                                out=hT[:, cols], in_=dps[:],
                                func=mybir.ActivationFunctionType.Relu,
                                bias=b1[:, :1])
                        else:
                            nc.vector.tensor_scalar(
                                out=outT[:, cols], in0=dps[:],
                                scalar1=b2[:, :1], scalar2=None,
                                op0=mybir.AluOpType.add)
                if layer == 0 and layers == 2:
                    # node-major h to DRAM, then exchange
                    for w in range(NW):
                        tps = tpp.tile([128, F], F32, tag="tp")
                        nc.tensor.transpose(
                            out=tps[:],
                            in_=hT[:, w * 128:(w + 1) * 128],
                            identity=ident[:HID, :HID],
                        )
                        hst = spool.tile([128, F], F32, tag="hst")
                        nc.vector.tensor_copy(out=hst[:], in_=tps[:])
                        rows = min(128, NPC - w * 128)
                        nc.sync.dma_start(
                            out=h_shard[w * 128:w * 128 + rows, :],
                            in_=hst[:rows, :])
                    if collective:
                        nc.gpsimd.collective_compute(
                            "AllGather",
                            mybir.AluOpType.bypass,
                            replica_groups=[list(range(NCORES))],
                            ins=[h_shard[:]],
                            outs=[h_full[:]],
                        )
                    else:
                        for w in range(0, NW, 8):
                            rows = min(1024, NPC - w * 128)
                            nc.sync.dma_start(
                                out=h_full[w * 128:w * 128 + rows, :],
                                in_=h_shard[w * 128:w * 128 + rows, :])
            nc.sync.dma_start(out=outT_d[:], in_=outT[:])
    nc.compile()
    return nc


_CACHE = {}


def _get_compiled(edge_index):
    key = edge_index.tobytes()[:4096] + str(edge_index.sum()).encode()
    if key not in _CACHE:
        plan, per_core = _plan_edges(edge_index)
        nc = _build(plan)
        _CACHE[key] = (nc, plan, per_core)
    return _CACHE[key]


def kernel(x, edge_index, W1_l, b1, W1_r, W2_l, b2, W2_r,
           _trace=False, _tmpdir=None):
    nc, plan, per_core = _get_compiled(edge_index)

    shared = {
        "x": np.ascontiguousarray(x.astype(np.float32)),
        "W1lT": np.ascontiguousarray(W1_l.T.astype(np.float32)),
        "W1rT": np.ascontiguousarray(W1_r.T.astype(np.float32)),
        "W2lT": np.ascontiguousarray(W2_l.T.astype(np.float32)),
        "W2rT": np.ascontiguousarray(W2_r.T.astype(np.float32)),
        "b1": np.ascontiguousarray(b1.reshape(HID, 1).astype(np.float32)),
        "b2": np.ascontiguousarray(b2.reshape(OUT, 1).astype(np.float32)),
        "iota": np.ascontiguousarray(
            np.tile(np.arange(128, dtype=np.float32)[None, :], (128, 1))),
    }
    in_maps = []
    for c in range(NCORES):
        xTc = np.zeros((F, NPC_PAD), np.float32)
        xTc[:, :NPC] = x[c * NPC:(c + 1) * NPC].T
        m = dict(shared)
        m["xT"] = xTc
        m.update(per_core[c])
        in_maps.append(m)

    res = run_bass_kernel_spmd(nc, in_maps, list(range(NCORES)),
                               trace=_trace, tmpdir=_tmpdir)
    out = np.empty((N, OUT), np.float32)
    for c in range(NCORES):
        out[c * NPC:(c + 1) * NPC] = res.results[c]["outT"][:, :NPC].T
    if _trace:
        return out, res
    return out



# revision 2
# speedup vs baseline: 1.2790x; 1.2790x over previous
"""GraphSAGE 2-layer forward on 8 Trainium2 NeuronCores.

Strategy (dst-range sharding, bf16 aggregation):
  - Core c owns destination nodes [c*NPC, (c+1)*NPC). It receives ALL edges
    whose dst lands in its range, so local segment-sums are exact (no
    all-reduce). One AllGather exchanges the hidden layer between layers.
  - Edges are host-sorted by destination into 128-node windows. Per window,
    messages are DMA-gathered from a bf16 row-padded table ([N,128]bf16 =
    256B rows, the dma_gather minimum element) into edge-major SBUF tiles.
    A weighted one-hot (iota==dstloc)*(1/deg) built in one DVE op turns the
    per-window segment-mean into bf16 PE matmuls (1 cycle/row vs 4 for f32)
    accumulating into PSUM (feature-major mean^T [64, 128 nodes]).
  - dma_gather indices are int16 (<32768), so edges are split into two
    passes: src < 32768 (table base 0) and src >= 32768 (table base shifted).
    Both passes accumulate into the same PSUM window.
  - Dense part: layer 1 in fp32 (h^T = relu(W_l @ mean^T + W_r @ x^T + b)),
    layer 2 in bf16 operands with fp32 PSUM. PE transpose writes node-major
    bf16 h (padded rows) to DRAM for the layer-2 gather.
"""

import numpy as np
import ml_dtypes

import concourse.bass as bass
import concourse.bacc as bacc
import concourse.tile as tile
from concourse import mybir
from concourse.bass_utils import run_bass_kernel_spmd
from concourse.masks import make_identity

F32 = mybir.dt.float32
BF16 = mybir.dt.bfloat16
I16 = mybir.dt.int16
NPBF16 = ml_dtypes.bfloat16

# Problem constants (hardcoded per contract)
N = 50000
E = 800000
F = 64
FP = 128                   # padded row width (256B bf16 rows for dma_gather)
HID = 64
OUT = 2
NCORES = 8
NPC = N // NCORES          # 6250 nodes per core
WIN = 128                  # nodes per window (one PSUM bank width)
NW = (NPC + WIN - 1) // WIN  # 49 windows per core
NPC_PAD = NW * WIN         # 6272
SPLIT = 32768              # int16 index limit
CHUNK_TILES = 96           # max message tiles gathered per chunk


def _plan_edges(edge_index):
    """Host-side graph preprocessing: per-core, per-window, per-pass edge
    slotting. Returns compile-time tile plan + per-core device arrays."""
    src = edge_index[0].astype(np.int64)
    dst = edge_index[1].astype(np.int64)
    deg = np.bincount(dst, minlength=N)
    wrec = (1.0 / np.maximum(deg, 1)).astype(np.float32)

    core = dst // NPC
    loc = dst % NPC
    win = loc // WIN
    locw = loc % WIN
    pas = (src >= SPLIT).astype(np.int64)

    key = (core * NW + win) * 2 + pas
    cnt = np.bincount(key, minlength=NCORES * NW * 2).reshape(NCORES, NW, 2)
    # tiles per (window, pass), shared across cores (same compiled program)
    tiles = np.maximum(1, -(-cnt.max(axis=0) // 128))  # [NW, 2]
    tA = tiles[:, 0]
    tB = tiles[:, 1]
    TA, TB = int(tA.sum()), int(tB.sum())
    a0 = np.concatenate([[0], np.cumsum(tA)])  # A-tile offsets per window
    b0 = np.concatenate([[0], np.cumsum(tB)])
    LA, LB = TA * 128, TB * 128

    order = np.argsort(key, kind="stable")

    per_core = []
    for c in range(NCORES):
        arrs = {}
        for p, (Tn, base, L) in enumerate(((tA, a0, LA), (tB, b0, LB))):
            idx_flat = np.zeros(L, np.int16)
            dl_flat = np.full(L, 210.0, np.float32)
            wv_flat = np.zeros(L, np.float32)
            for w in range(NW):
                k = (c * NW + w) * 2 + p
                s0 = int(np.searchsorted(key[order], k))
                s1 = int(np.searchsorted(key[order], k + 1))
                ed = order[s0:s1]
                j = base[w] * 128 + np.arange(len(ed))
                sv = src[ed] - (SPLIT if p else 0)
                idx_flat[j] = sv.astype(np.int16)
                dl_flat[j] = locw[ed].astype(np.float32)
                wv_flat[j] = wrec[dst[ed]]
            tag = "AB"[p]
            arrs[f"idx{tag}"] = np.ascontiguousarray(
                np.tile(idx_flat.reshape(L // 16, 16).T, (8, 1)))
            arrs[f"dl{tag}"] = np.ascontiguousarray(
                dl_flat.reshape(-1, 128).T)
            arrs[f"wv{tag}"] = np.ascontiguousarray(
                wv_flat.reshape(-1, 128).T)
        per_core.append(arrs)

    # chunk windows so that each chunk's message tiles fit SBUF
    chunks = []
    cur = []
    cur_t = 0
    for w in range(NW):
        t = int(tA[w] + tB[w])
        if cur and cur_t + t > CHUNK_TILES:
            chunks.append(cur)
            cur = []
            cur_t = 0
        cur.append(w)
        cur_t += t
    if cur:
        chunks.append(cur)

    plan = dict(tA=tA.tolist(), tB=tB.tolist(),
                a0=a0.tolist(), b0=b0.tolist(),
                TA=TA, TB=TB, chunks=chunks)
    return plan, per_core


def _build(plan, collective=True, layers=2):
    """Build the SPMD Bass program (same for all cores)."""
    tA, tB = plan["tA"], plan["tB"]
    a0, b0 = plan["a0"], plan["b0"]
    TA, TB = plan["TA"], plan["TB"]
    chunks = plan["chunks"]
    maxA = max(sum(tA[w] for w in ch) for ch in chunks)
    maxB = max(sum(tB[w] for w in ch) for ch in chunks)

    nc = bacc.Bacc("TRN2", target_bir_lowering=False, debug=False,
                   num_devices=NCORES)

    xpad_d = nc.dram_tensor("xpad", [N, FP], BF16, kind="ExternalInput")
    xT_d = nc.dram_tensor("xT", [F, NPC_PAD], F32, kind="ExternalInput")
    idxA_d = nc.dram_tensor("idxA", [128, TA * 8], I16, kind="ExternalInput")
    idxB_d = nc.dram_tensor("idxB", [128, TB * 8], I16, kind="ExternalInput")
    dlA_d = nc.dram_tensor("dlA", [128, TA], F32, kind="ExternalInput")
    wvA_d = nc.dram_tensor("wvA", [128, TA], F32, kind="ExternalInput")
    dlB_d = nc.dram_tensor("dlB", [128, TB], F32, kind="ExternalInput")
    wvB_d = nc.dram_tensor("wvB", [128, TB], F32, kind="ExternalInput")
    w1l_d = nc.dram_tensor("W1lT", [F, HID], F32, kind="ExternalInput")
    w1r_d = nc.dram_tensor("W1rT", [F, HID], F32, kind="ExternalInput")
    w2l_d = nc.dram_tensor("W2lT", [HID, OUT], BF16, kind="ExternalInput")
    w2r_d = nc.dram_tensor("W2rT", [HID, OUT], BF16, kind="ExternalInput")
    b1_d = nc.dram_tensor("b1", [HID, 1], F32, kind="ExternalInput")
    b2_d = nc.dram_tensor("b2", [OUT, 1], F32, kind="ExternalInput")
    iota_d = nc.dram_tensor("iota", [128, 128], BF16, kind="ExternalInput")
    outT_d = nc.dram_tensor("outT", [OUT, NPC_PAD], F32, kind="ExternalOutput")

    h_shard = nc.dram_tensor("h_shard", [NPC, FP], BF16)
    h_full = nc.dram_tensor("h_full", [N, FP], BF16,
                            addr_space="Shared" if collective else "Local")

    with tile.TileContext(nc) as tc:
        with (
            tc.tile_pool(name="const", bufs=1) as cpool,
            tc.tile_pool(name="msg", bufs=2) as mpool,
            tc.tile_pool(name="oh", bufs=4) as ohpool,
            tc.tile_pool(name="small", bufs=3) as spool,
            tc.tile_pool(name="agg", bufs=4, space="PSUM") as aggp,
            tc.tile_pool(name="dense", bufs=2, space="PSUM") as densep,
            tc.tile_pool(name="tp", bufs=2, space="PSUM") as tpp,
        ):
            # ---- constants to SBUF
            iota = cpool.tile([128, 128], BF16)
            nc.sync.dma_start(out=iota[:], in_=iota_d[:])
            ident = cpool.tile([128, 128], BF16)
            make_identity(nc, ident[:])
            w1l = cpool.tile([F, HID], F32)
            nc.sync.dma_start(out=w1l[:], in_=w1l_d[:])
            w1r = cpool.tile([F, HID], F32)
            nc.sync.dma_start(out=w1r[:], in_=w1r_d[:])
            w2l = cpool.tile([HID, OUT], BF16)
            nc.sync.dma_start(out=w2l[:], in_=w2l_d[:])
            w2r = cpool.tile([HID, OUT], BF16)
            nc.sync.dma_start(out=w2r[:], in_=w2r_d[:])
            b1 = cpool.tile([HID, 1], F32)
            nc.sync.dma_start(out=b1[:], in_=b1_d[:])
            b2 = cpool.tile([OUT, 1], F32)
            nc.sync.dma_start(out=b2[:], in_=b2_d[:])
            xT = cpool.tile([F, NPC_PAD], F32)
            nc.sync.dma_start(out=xT[:], in_=xT_d[:])
            idxA = cpool.tile([128, TA * 8], I16)
            nc.sync.dma_start(out=idxA[:], in_=idxA_d[:])
            idxB = cpool.tile([128, TB * 8], I16)
            nc.sync.dma_start(out=idxB[:], in_=idxB_d[:])
            dlA = cpool.tile([128, TA], F32)
            nc.sync.dma_start(out=dlA[:], in_=dlA_d[:])
            wvA = cpool.tile([128, TA], F32)
            nc.sync.dma_start(out=wvA[:], in_=wvA_d[:])
            dlB = cpool.tile([128, TB], F32)
            nc.sync.dma_start(out=dlB[:], in_=dlB_d[:])
            wvB = cpool.tile([128, TB], F32)
            nc.sync.dma_start(out=wvB[:], in_=wvB_d[:])

            hTb = cpool.tile([HID, NPC_PAD], BF16)
            outT = cpool.tile([OUT, NPC_PAD], F32)

            for layer in range(layers):
                for ch in chunks:
                    w_lo, w_hi = ch[0], ch[-1] + 1
                    ca0, ca1 = a0[w_lo], a0[w_hi]
                    cb0, cb1 = b0[w_lo], b0[w_hi]
                    nta, ntb = ca1 - ca0, cb1 - cb0
                    msgA = mpool.tile([128, maxA * FP], BF16, tag="msgA")
                    msgB = mpool.tile([128, maxB * FP], BF16, tag="msgB")
                    for (msg, nt, cc0, idx, base) in (
                        (msgA, nta, ca0, idxA, 0),
                        (msgB, ntb, cb0, idxB, SPLIT),
                    ):
                        table = xpad_d if layer == 0 else h_full
                        nc.gpsimd.dma_gather(
                            out_ap=msg[:, :nt * FP].rearrange(
                                "p (t f) -> p t f", f=FP),
                            in_ap=table[base:, :],
                            idxs_ap=idx[:, cc0 * 8:(cc0 + nt) * 8],
                            num_idxs=nt * 128,
                            num_idxs_reg=nt * 128,
                            elem_size=FP,
                            single_packet=False,
                        )
                    for w in ch:
                        psum = aggp.tile([F, 128], F32, tag="agg")
                        work = (
                            [(msgA, t - ca0, dlA, wvA, t)
                             for t in range(a0[w], a0[w + 1])]
                            + [(msgB, t - cb0, dlB, wvB, t)
                               for t in range(b0[w], b0[w + 1])]
                        )
                        for i, (msg, mc, dl, wv, t) in enumerate(work):
                            oh = ohpool.tile([128, 128], BF16, tag="oh")
                            nc.vector.tensor_scalar(
                                out=oh[:],
                                in0=iota[:],
                                scalar1=dl[:, t:t + 1],
                                scalar2=wv[:, t:t + 1],
                                op0=mybir.AluOpType.is_equal,
                                op1=mybir.AluOpType.mult,
                            )
                            nc.tensor.matmul(
                                out=psum[:],
                                lhsT=msg[:, mc * FP:mc * FP + F],
                                rhs=oh[:],
                                start=(i == 0),
                                stop=(i == len(work) - 1),
                            )
                        odim = HID if layer == 0 else OUT
                        dps = densep.tile([odim, 128], F32, tag="dense")
                        cols = slice(w * 128, (w + 1) * 128)
                        if layer == 0:
                            meanT = spool.tile([F, 128], F32, tag="meanT")
                            nc.vector.tensor_copy(out=meanT[:], in_=psum[:])
                            nc.tensor.matmul(out=dps[:], lhsT=w1l[:],
                                             rhs=meanT[:],
                                             start=True, stop=False)
                            nc.tensor.matmul(out=dps[:], lhsT=w1r[:],
                                             rhs=xT[:, cols],
                                             start=False, stop=True)
                            nc.scalar.activation(
                                out=hTb[:, cols], in_=dps[:],
                                func=mybir.ActivationFunctionType.Relu,
                                bias=b1[:, :1])
                        else:
                            meanT = spool.tile([F, 128], BF16, tag="meanTb")
                            nc.vector.tensor_copy(out=meanT[:], in_=psum[:])
                            nc.tensor.matmul(out=dps[:], lhsT=w2l[:],
                                             rhs=meanT[:],
                                             start=True, stop=False)
                            nc.tensor.matmul(out=dps[:], lhsT=w2r[:],
                                             rhs=hTb[:, cols],
                                             start=False, stop=True)
                            nc.vector.tensor_scalar(
                                out=outT[:, cols], in0=dps[:],
                                scalar1=b2[:, :1], scalar2=None,
                                op0=mybir.AluOpType.add)
                if layer == 0 and layers == 2:
                    # node-major bf16 h rows (64 real cols of 128) to DRAM
                    for w in range(NW):
                        tps = tpp.tile([128, F], BF16, tag="tp")
                        nc.tensor.transpose(
                            out=tps[:],
                            in_=hTb[:, w * 128:(w + 1) * 128],
                            identity=ident[:HID, :HID],
                        )
                        hst = spool.tile([128, F], BF16, tag="hst")
                        nc.vector.tensor_copy(out=hst[:], in_=tps[:])
                        rows = min(128, NPC - w * 128)
                        nc.sync.dma_start(
                            out=h_shard[w * 128:w * 128 + rows, :F],
                            in_=hst[:rows, :])
                    if collective:
                        nc.gpsimd.collective_compute(
                            "AllGather",
                            mybir.AluOpType.bypass,
                            replica_groups=[list(range(NCORES))],
                            ins=[h_shard[:]],
                            outs=[h_full[:]],
                        )
                    else:
                        for w in range(0, NW, 8):
                            rows = min(1024, NPC - w * 128)
                            nc.sync.dma_start(
                                out=h_full[w * 128:w * 128 + rows, :],
                                in_=h_shard[w * 128:w * 128 + rows, :])
            nc.sync.dma_start(out=outT_d[:], in_=outT[:])
    nc.compile()
    return nc


_CACHE = {}


def _get_compiled(edge_index):
    key = edge_index.tobytes()[:4096] + str(edge_index.sum()).encode()
    if key not in _CACHE:
        plan, per_core = _plan_edges(edge_index)
        nc = _build(plan)
        _CACHE[key] = (nc, plan, per_core)
    return _CACHE[key]


def kernel(x, edge_index, W1_l, b1, W1_r, W2_l, b2, W2_r,
           _trace=False, _tmpdir=None):
    nc, plan, per_core = _get_compiled(edge_index)

    xpad = np.zeros((N, FP), NPBF16)
    xpad[:, :F] = x.astype(NPBF16)
    shared = {
        "xpad": xpad,
        "W1lT": np.ascontiguousarray(W1_l.T.astype(np.float32)),
        "W1rT": np.ascontiguousarray(W1_r.T.astype(np.float32)),
        "W2lT": np.ascontiguousarray(W2_l.T.astype(NPBF16)),
        "W2rT": np.ascontiguousarray(W2_r.T.astype(NPBF16)),
        "b1": np.ascontiguousarray(b1.reshape(HID, 1).astype(np.float32)),
        "b2": np.ascontiguousarray(b2.reshape(OUT, 1).astype(np.float32)),
        "iota": np.ascontiguousarray(np.tile(
            np.arange(128, dtype=np.float32)[None, :], (128, 1))
        ).astype(NPBF16),
    }
    in_maps = []
    for c in range(NCORES):
        xTc = np.zeros((F, NPC_PAD), np.float32)
        xTc[:, :NPC] = x[c * NPC:(c + 1) * NPC].T
        m = dict(shared)
        m["xT"] = xTc
        m.update(per_core[c])
        in_maps.append(m)

    res = run_bass_kernel_spmd(nc, in_maps, list(range(NCORES)),
                               trace=_trace, tmpdir=_tmpdir)
    out = np.empty((N, OUT), np.float32)
    for c in range(NCORES):
        out[c * NPC:(c + 1) * NPC] = res.results[c]["outT"][:, :NPC].T
    if _trace:
        return out, res
    return out
